# revision 9
# baseline (speedup 1.0000x reference)
"""ContactNet grasp-frame kernel for Trainium2 (Bass/Tile, 8-core SPMD).

Computes, for N=4M points (sharded 8 ways along N):
    b      = z1 / ||z1||                      (col_x)
    inner  = b . z2
    perp   = z2 - inner * b
    col_z  = perp / ||perp||                  (== unit(approach_dirs))
    col_y  = (z2 x b) / ||perp||              (== cross(col_z, col_x), unit)
    grasp  = [[col_x col_y col_z contact], [0 0 0 1]]   (4x4, rows-major)
    s, w   = sigmoid(s_logits), sigmoid(w_logits)
plus a straight copy of contact_pts as the "points" output.

Layout strategy: all tensors stay interleaved ([pt, comp]) in SBUF; every
vector-engine op reads/writes component planes through strided access
patterns (free on DVE at fp32 1x rate), so no separate de/interleave passes
exist. Final multiplies write straight into the [128, 16F] grasp tile.
rsqrt = vector.reciprocal_approx_fast + scalar.sqrt (ACT Rsqrt is banned
for accuracy). Squares, sigmoid, and the t-column copy run on ScalarE;
constants on GpSimd; everything else on VectorE.
"""

import numpy as np

import concourse.bass as bass
import concourse.bacc as bacc
import concourse.mybir as mybir
from concourse.tile import TileContext
from concourse.bass_utils import run_bass_kernel_spmd

N_CORES = 8
N_TOTAL = 4194304
M_CORE = N_TOTAL // N_CORES  # 524288 points per core
P = 128                      # SBUF partitions
F = 512                      # points per partition per tile
C = P * F                    # points per tile

f32 = mybir.dt.float32


import contextlib


def _nullctx():
    return contextlib.nullcontext()


def build_nc(m=M_CORE, repeats=1):
    """Build the single-core Bass program for m points (SPMD across cores).

    repeats > 1 wraps the whole compute in a For_i loop that re-runs it
    (same I/O, idempotent) — used only for wall-clock-delta benchmarking.
    """
    t = m // C
    assert t * C == m

    nc = bacc.Bacc()
    z1 = nc.declare_dram_parameter("z1", [m, 3], f32, isOutput=False)
    z2 = nc.declare_dram_parameter("z2", [m, 3], f32, isOutput=False)
    cp = nc.declare_dram_parameter("cp", [m, 3], f32, isOutput=False)
    sl = nc.declare_dram_parameter("sl", [m], f32, isOutput=False)
    wl = nc.declare_dram_parameter("wl", [m], f32, isOutput=False)
    gr = nc.declare_dram_parameter("gr", [m, 16], f32, isOutput=True)
    pt = nc.declare_dram_parameter("pt", [m, 3], f32, isOutput=True)
    so = nc.declare_dram_parameter("so", [m], f32, isOutput=True)
    wo = nc.declare_dram_parameter("wo", [m], f32, isOutput=True)

    # DRAM tile views: [t, 128, F*comps]
    z1v = z1.rearrange("(t p f) c -> t p (f c)", t=t, p=P, f=F)
    z2v = z2.rearrange("(t p f) c -> t p (f c)", t=t, p=P, f=F)
    cpv = cp.rearrange("(t p f) c -> t p (f c)", t=t, p=P, f=F)
    slv = sl.rearrange("(t p f) -> t p f", t=t, p=P, f=F)
    wlv = wl.rearrange("(t p f) -> t p f", t=t, p=P, f=F)
    grv = gr.rearrange("(t p f) c -> t p (f c)", t=t, p=P, f=F)
    ptv = pt.rearrange("(t p f) c -> t p (f c)", t=t, p=P, f=F)
    sov = so.rearrange("(t p f) -> t p f", t=t, p=P, f=F)
    wov = wo.rearrange("(t p f) -> t p f", t=t, p=P, f=F)

    Sq = mybir.ActivationFunctionType.Square
    Sqrt = mybir.ActivationFunctionType.Sqrt
    Sig = mybir.ActivationFunctionType.Sigmoid
    Cpy = mybir.ActivationFunctionType.Copy

    def bc3(ap):  # [P, F] -> [P, F, 3] stride-0 broadcast
        return ap.unsqueeze(2).broadcast_to([P, F, 3])

    with TileContext(nc) as tc:
        with (
            tc.tile_pool(name="io", bufs=2) as io,
            tc.tile_pool(name="v3", bufs=2) as v3,
            tc.tile_pool(name="sc", bufs=2) as sc,
            tc.tile_pool(name="go", bufs=2) as go,
            tc.For_i(0, repeats, 1) if repeats > 1 else _nullctx(),
        ):
            for i in range(t):
                z1t = io.tile([P, 3 * F], f32, tag="z1t")
                z2t = io.tile([P, 3 * F], f32, tag="z2t")
                cpt = io.tile([P, 3 * F], f32, tag="cpt")
                slt = io.tile([P, F], f32, tag="slt")
                wlt = io.tile([P, F], f32, tag="wlt")
                nc.sync.dma_start(out=z1t[:, :], in_=z1v[i])
                nc.sync.dma_start(out=z2t[:, :], in_=z2v[i])
                nc.sync.dma_start(out=cpt[:, :], in_=cpv[i])
                nc.sync.dma_start(out=slt[:, :], in_=slv[i])
                nc.sync.dma_start(out=wlt[:, :], in_=wlv[i])

                z1c = z1t[:, :].rearrange("p (f c) -> p f c", c=3)
                z2c = z2t[:, :].rearrange("p (f c) -> p f c", c=3)
                cpc = cpt[:, :].rearrange("p (f c) -> p f c", c=3)

                g = go.tile([P, 16 * F], f32, tag="g")
                gv = g[:, :].rearrange("p (f r c) -> p f r c", r=4, c=4)
                bcol = gv[:, :, 0:3, 0]   # col_x slots
                ycol = gv[:, :, 0:3, 1]   # col_y slots
                zcol = gv[:, :, 0:3, 2]   # col_z slots
                tcol = gv[:, :, 0:3, 3]   # translation slots

                # ---- d11 = z1.z1 ; r11 = 1/sqrt(d11) ----
                sq1 = v3.tile([P, 3 * F], f32, tag="sq")
                nc.scalar.activation(sq1[:, :], z1t[:, :], Sq)
                sq1c = sq1[:, :].rearrange("p (f c) -> p f c", c=3)
                ta = sc.tile([P, F], f32, tag="ta")
                d11 = sc.tile([P, F], f32, tag="d11")
                nc.vector.tensor_add(ta[:, :], sq1c[:, :, 0], sq1c[:, :, 1])
                nc.vector.tensor_add(d11[:, :], ta[:, :], sq1c[:, :, 2])
                inv1 = sc.tile([P, F], f32, tag="inv1")
                rscr = sc.tile([P, F], f32, tag="rscr")
                nc.vector.reciprocal_approx_accurate(inv1[:, :], d11[:, :], rscr[:, :])
                r11 = sc.tile([P, F], f32, tag="r11")
                nc.scalar.activation(r11[:, :], inv1[:, :], Sqrt)

                # ---- b = z1 * r11  -> grasp col 0 ----
                nc.vector.tensor_mul(bcol, z1c, bc3(r11[:, :]))

                # ---- inner = b . z2 ----
                pr = v3.tile([P, 3 * F], f32, tag="pr")
                prc = pr[:, :].rearrange("p (f c) -> p f c", c=3)
                nc.vector.tensor_mul(prc, bcol, z2c)
                tb = sc.tile([P, F], f32, tag="tb")
                inner = sc.tile([P, F], f32, tag="inner")
                nc.vector.tensor_add(tb[:, :], prc[:, :, 0], prc[:, :, 1])
                nc.vector.tensor_add(inner[:, :], tb[:, :], prc[:, :, 2])

                # ---- perp = z2 - inner * b ----
                perp = v3.tile([P, 3 * F], f32, tag="perp")
                pc = perp[:, :].rearrange("p (f c) -> p f c", c=3)
                nc.vector.tensor_mul(pc, bc3(inner[:, :]), bcol)
                nc.vector.tensor_sub(pc, z2c, pc)

                # ---- dpp = perp.perp ; rp = 1/sqrt(dpp) ----
                sqp = v3.tile([P, 3 * F], f32, tag="sq")
                nc.scalar.activation(sqp[:, :], perp[:, :], Sq)
                sqpc = sqp[:, :].rearrange("p (f c) -> p f c", c=3)
                tc2 = sc.tile([P, F], f32, tag="tc2")
                dpp = sc.tile([P, F], f32, tag="dpp")
                nc.vector.tensor_add(tc2[:, :], sqpc[:, :, 0], sqpc[:, :, 1])
                nc.vector.tensor_add(dpp[:, :], tc2[:, :], sqpc[:, :, 2])
                invp = sc.tile([P, F], f32, tag="invp")
                nc.vector.reciprocal_approx_fast(invp[:, :], dpp[:, :])
                rp = sc.tile([P, F], f32, tag="rp")
                nc.scalar.activation(rp[:, :], invp[:, :], Sqrt)

                # ---- col_z = perp * rp -> grasp col 2 ----
                nc.vector.tensor_mul(zcol, pc, bc3(rp[:, :]))

                # ---- cross c = z2 x b -> grasp col 1 (raw), then *= rp ----
                m1 = sc.tile([P, F], f32, tag="mA")
                m2 = sc.tile([P, F], f32, tag="mB")
                nc.vector.tensor_mul(m1[:, :], z2c[:, :, 1], gv[:, :, 2, 0])
                nc.vector.tensor_mul(m2[:, :], z2c[:, :, 2], gv[:, :, 1, 0])
                nc.vector.tensor_sub(gv[:, :, 0, 1], m1[:, :], m2[:, :])
                m3 = sc.tile([P, F], f32, tag="mA")
                m4 = sc.tile([P, F], f32, tag="mB")
                nc.vector.tensor_mul(m3[:, :], z2c[:, :, 2], gv[:, :, 0, 0])
                nc.vector.tensor_mul(m4[:, :], z2c[:, :, 0], gv[:, :, 2, 0])
                nc.vector.tensor_sub(gv[:, :, 1, 1], m3[:, :], m4[:, :])
                m5 = sc.tile([P, F], f32, tag="mA")
                m6 = sc.tile([P, F], f32, tag="mB")
                nc.vector.tensor_mul(m5[:, :], z2c[:, :, 0], gv[:, :, 1, 0])
                nc.vector.tensor_mul(m6[:, :], z2c[:, :, 1], gv[:, :, 0, 0])
                nc.vector.tensor_sub(gv[:, :, 2, 1], m5[:, :], m6[:, :])
                nc.vector.tensor_mul(ycol, ycol, bc3(rp[:, :]))

                # ---- t column + constant bottom row ----
                nc.scalar.activation(tcol, cpc, Cpy)
                nc.gpsimd.memset(gv[:, :, 3, 0:3], 0.0)
                nc.gpsimd.memset(gv[:, :, 3, 3], 1.0)

                # ---- sigmoids ----
                sot = io.tile([P, F], f32, tag="sot")
                wot = io.tile([P, F], f32, tag="wot")
                nc.scalar.activation(sot[:, :], slt[:, :], Sig)
                nc.scalar.activation(wot[:, :], wlt[:, :], Sig)

                # ---- stores ----
                nc.sync.dma_start(out=grv[i], in_=g[:, :])
                nc.sync.dma_start(out=ptv[i], in_=cpt[:, :])
                nc.sync.dma_start(out=sov[i], in_=sot[:, :])
                nc.sync.dma_start(out=wov[i], in_=wot[:, :])
    nc.finalize()
    return nc


_NC_CACHE = {}


def _get_nc(m):
    if m not in _NC_CACHE:
        _NC_CACHE[m] = build_nc(m)
    return _NC_CACHE[m]


def kernel(contact_pts, z1, z2, s_logits, w_logits, num_batches, _trace=False):
    n = contact_pts.shape[0]
    m = n // N_CORES
    nc = _get_nc(m)

    cp = np.ascontiguousarray(contact_pts, dtype=np.float32)
    z1 = np.ascontiguousarray(z1, dtype=np.float32)
    z2 = np.ascontiguousarray(z2, dtype=np.float32)
    sl = np.ascontiguousarray(s_logits, dtype=np.float32).reshape(n)
    wl = np.ascontiguousarray(w_logits, dtype=np.float32).reshape(n)

    in_maps = []
    for c in range(N_CORES):
        s0, s1 = c * m, (c + 1) * m
        in_maps.append({
            "z1": z1[s0:s1],
            "z2": z2[s0:s1],
            "cp": cp[s0:s1],
            "sl": sl[s0:s1],
            "wl": wl[s0:s1],
        })

    res = run_bass_kernel_spmd(nc, in_maps, list(range(N_CORES)), trace=_trace)

    points = np.concatenate([res.results[c]["pt"] for c in range(N_CORES)], axis=0)
    grasps = np.concatenate([res.results[c]["gr"] for c in range(N_CORES)], axis=0)
    s = np.concatenate([res.results[c]["so"] for c in range(N_CORES)], axis=0)
    w = np.concatenate([res.results[c]["wo"] for c in range(N_CORES)], axis=0)

    b = int(num_batches)
    p = n // b
    out = (
        points.reshape(b, p, 3),
        grasps.reshape(b, p, 4, 4),
        s.reshape(b, p),
        w.reshape(b, p),
    )
    if _trace:
        return out, res
    return out


# revision 11
# speedup vs baseline: 1.2969x; 1.2969x over previous
"""ContactNet grasp-frame kernel for Trainium2 (Bass/Tile, 8-core SPMD).

Computes, for N=4M points (sharded 8 ways along N):
    b      = z1 / ||z1||                      (col_x)
    inner  = b . z2
    perp   = z2 - inner * b
    col_z  = perp / ||perp||                  (== unit(approach_dirs))
    col_y  = (z2 x b) / ||perp||              (== cross(col_z, col_x), unit)
    grasp  = [[col_x col_y col_z contact], [0 0 0 1]]   (4x4, rows-major)
    s, w   = sigmoid(s_logits), sigmoid(w_logits)
plus a straight copy of contact_pts as the "points" output.

Layout strategy: all tensors stay interleaved ([pt, comp]) in SBUF; every
vector-engine op reads/writes component planes through strided access
patterns (free on DVE at fp32 1x rate), so no separate de/interleave passes
exist. Final multiplies write straight into the [128, 16F] grasp tile.
rsqrt = vector.reciprocal_approx_fast + scalar.sqrt (ACT Rsqrt is banned
for accuracy). Squares, sigmoid, and the t-column copy run on ScalarE;
constants on GpSimd; everything else on VectorE.
"""

import numpy as np

import concourse.bass as bass
import concourse.bacc as bacc
import concourse.mybir as mybir
from concourse.tile import TileContext
from concourse.bass_utils import run_bass_kernel_spmd

N_CORES = 8
N_TOTAL = 4194304
M_CORE = N_TOTAL // N_CORES  # 524288 points per core
P = 128                      # SBUF partitions
F = 512                      # points per partition per tile
C = P * F                    # points per tile

f32 = mybir.dt.float32


import contextlib


def _nullctx():
    return contextlib.nullcontext()


def build_nc(m=M_CORE, repeats=1):
    """Build the single-core Bass program for m points (SPMD across cores).

    repeats > 1 wraps the whole compute in a For_i loop that re-runs it
    (same I/O, idempotent) — used only for wall-clock-delta benchmarking.
    """
    t = m // C
    assert t * C == m

    nc = bacc.Bacc()
    z1 = nc.declare_dram_parameter("z1", [m, 3], f32, isOutput=False)
    z2 = nc.declare_dram_parameter("z2", [m, 3], f32, isOutput=False)
    cp = nc.declare_dram_parameter("cp", [m, 3], f32, isOutput=False)
    sl = nc.declare_dram_parameter("sl", [m], f32, isOutput=False)
    wl = nc.declare_dram_parameter("wl", [m], f32, isOutput=False)
    gr = nc.declare_dram_parameter("gr", [m, 16], f32, isOutput=True)
    pt = nc.declare_dram_parameter("pt", [m, 3], f32, isOutput=True)
    so = nc.declare_dram_parameter("so", [m], f32, isOutput=True)
    wo = nc.declare_dram_parameter("wo", [m], f32, isOutput=True)

    # DRAM tile views: [t, 128, F*comps]
    z1v = z1.rearrange("(t p f) c -> t p (f c)", t=t, p=P, f=F)
    z2v = z2.rearrange("(t p f) c -> t p (f c)", t=t, p=P, f=F)
    cpv = cp.rearrange("(t p f) c -> t p (f c)", t=t, p=P, f=F)
    slv = sl.rearrange("(t p f) -> t p f", t=t, p=P, f=F)
    wlv = wl.rearrange("(t p f) -> t p f", t=t, p=P, f=F)
    grv = gr.rearrange("(t p f) c -> t p (f c)", t=t, p=P, f=F)
    ptv = pt.rearrange("(t p f) c -> t p (f c)", t=t, p=P, f=F)
    sov = so.rearrange("(t p f) -> t p f", t=t, p=P, f=F)
    wov = wo.rearrange("(t p f) -> t p f", t=t, p=P, f=F)

    Sq = mybir.ActivationFunctionType.Square
    Sqrt = mybir.ActivationFunctionType.Sqrt
    Sig = mybir.ActivationFunctionType.Sigmoid
    Cpy = mybir.ActivationFunctionType.Copy

    def bc3(ap):  # [P, F] -> [P, F, 3] stride-0 broadcast
        return ap.unsqueeze(2).broadcast_to([P, F, 3])

    with TileContext(nc) as tc:
        with (
            tc.tile_pool(name="io", bufs=2) as io,
            tc.tile_pool(name="v3", bufs=2) as v3,
            tc.tile_pool(name="sc", bufs=2) as sc,
            tc.tile_pool(name="sg", bufs=2) as sg,
            tc.tile_pool(name="go", bufs=1) as go,
        ):
            # Persistent double-buffered grasp tiles: the constant bottom row
            # [0,0,0,1] is written once here and survives buffer reuse (each
            # iteration only overwrites columns 0-11 of every 16).
            gbufs = []
            for j in range(2):
                gb = go.tile([P, 16 * F], f32, tag=f"g{j}", name=f"gbuf{j}")
                gbv = gb[:, :].rearrange("p (f r c) -> p f r c", r=4, c=4)
                nc.gpsimd.memset(gbv[:, :, 3, 0:3], 0.0)
                nc.gpsimd.memset(gbv[:, :, 3, 3], 1.0)
                gbufs.append(gb)

            with tc.For_i(0, repeats, 1) if repeats > 1 else _nullctx():
                # ---- sigmoid phase first: keeps the main loop inside one
                # ACT table set (sqrt_and_others: Square/Sqrt/Copy) ----
                for i in range(t):
                    slt = sg.tile([P, F], f32, tag="slt")
                    wlt = sg.tile([P, F], f32, tag="wlt")
                    nc.sync.dma_start(out=slt[:, :], in_=slv[i])
                    nc.sync.dma_start(out=wlt[:, :], in_=wlv[i])
                    sot = sg.tile([P, F], f32, tag="sot")
                    wot = sg.tile([P, F], f32, tag="wot")
                    nc.scalar.activation(sot[:, :], slt[:, :], Sig)
                    nc.scalar.activation(wot[:, :], wlt[:, :], Sig)
                    nc.sync.dma_start(out=sov[i], in_=sot[:, :])
                    nc.sync.dma_start(out=wov[i], in_=wot[:, :])

                for i in range(t):
                    z1t = io.tile([P, 3 * F], f32, tag="z1t")
                    z2t = io.tile([P, 3 * F], f32, tag="z2t")
                    cpt = io.tile([P, 3 * F], f32, tag="cpt")
                    nc.sync.dma_start(out=z1t[:, :], in_=z1v[i])
                    nc.sync.dma_start(out=z2t[:, :], in_=z2v[i])
                    nc.sync.dma_start(out=cpt[:, :], in_=cpv[i])

                    z1c = z1t[:, :].rearrange("p (f c) -> p f c", c=3)
                    z2c = z2t[:, :].rearrange("p (f c) -> p f c", c=3)
                    cpc = cpt[:, :].rearrange("p (f c) -> p f c", c=3)

                    g = gbufs[i % 2]
                    gv = g[:, :].rearrange("p (f r c) -> p f r c", r=4, c=4)
                    bcol = gv[:, :, 0:3, 0]   # col_x slots
                    ycol = gv[:, :, 0:3, 1]   # col_y slots
                    zcol = gv[:, :, 0:3, 2]   # col_z slots
                    tcol = gv[:, :, 0:3, 3]   # translation slots

                    # ---- d11 = z1.z1 ; r11 = 1/sqrt(d11) ----
                    sq1 = v3.tile([P, 3 * F], f32, tag="sq")
                    nc.scalar.activation(sq1[:, :], z1t[:, :], Sq)
                    sq1c = sq1[:, :].rearrange("p (f c) -> p f c", c=3)
                    ta = sc.tile([P, F], f32, tag="tA", bufs=4)
                    d11 = sc.tile([P, F], f32, tag="dot", bufs=4)
                    nc.vector.tensor_add(ta[:, :], sq1c[:, :, 0], sq1c[:, :, 1])
                    nc.vector.tensor_add(d11[:, :], ta[:, :], sq1c[:, :, 2])
                    inv1 = sc.tile([P, F], f32, tag="inv")
                    rscr = sc.tile([P, F], f32, tag="rscr")
                    nc.vector.reciprocal_approx_accurate(
                        inv1[:, :], d11[:, :], rscr[:, :])
                    r11 = sc.tile([P, F], f32, tag="rs", bufs=4)
                    nc.scalar.activation(r11[:, :], inv1[:, :], Sqrt)

                    # ---- b = z1 * r11  -> grasp col 0 ----
                    nc.vector.tensor_mul(bcol, z1c, bc3(r11[:, :]))

                    # ---- inner = b . z2 ----
                    pr = v3.tile([P, 3 * F], f32, tag="pr")
                    prc = pr[:, :].rearrange("p (f c) -> p f c", c=3)
                    nc.vector.tensor_mul(prc, bcol, z2c)
                    tb = sc.tile([P, F], f32, tag="tA", bufs=4)
                    inner = sc.tile([P, F], f32, tag="dot", bufs=4)
                    nc.vector.tensor_add(tb[:, :], prc[:, :, 0], prc[:, :, 1])
                    nc.vector.tensor_add(inner[:, :], tb[:, :], prc[:, :, 2])

                    # ---- perp = z2 - inner * b ----
                    perp = v3.tile([P, 3 * F], f32, tag="perp")
                    pc = perp[:, :].rearrange("p (f c) -> p f c", c=3)
                    nc.vector.tensor_mul(pc, bc3(inner[:, :]), bcol)
                    nc.vector.tensor_sub(pc, z2c, pc)

                    # ---- dpp = perp.perp ; rp = 1/sqrt(dpp) ----
                    sqp = v3.tile([P, 3 * F], f32, tag="sq")
                    nc.scalar.activation(sqp[:, :], perp[:, :], Sq)
                    sqpc = sqp[:, :].rearrange("p (f c) -> p f c", c=3)
                    tc2 = sc.tile([P, F], f32, tag="tA", bufs=4)
                    dpp = sc.tile([P, F], f32, tag="dot", bufs=4)
                    nc.vector.tensor_add(tc2[:, :], sqpc[:, :, 0], sqpc[:, :, 1])
                    nc.vector.tensor_add(dpp[:, :], tc2[:, :], sqpc[:, :, 2])
                    invp = sc.tile([P, F], f32, tag="inv")
                    nc.vector.reciprocal_approx_fast(invp[:, :], dpp[:, :])
                    rp = sc.tile([P, F], f32, tag="rs", bufs=4)
                    nc.scalar.activation(rp[:, :], invp[:, :], Sqrt)

                    # ---- col_z = perp * rp -> grasp col 2 ----
                    nc.vector.tensor_mul(zcol, pc, bc3(rp[:, :]))

                    # ---- cross c = z2 x b -> grasp col 1 (raw), then *= rp ----
                    m1 = sc.tile([P, F], f32, tag="mA")
                    m2 = sc.tile([P, F], f32, tag="mB")
                    nc.vector.tensor_mul(m1[:, :], z2c[:, :, 1], gv[:, :, 2, 0])
                    nc.vector.tensor_mul(m2[:, :], z2c[:, :, 2], gv[:, :, 1, 0])
                    nc.vector.tensor_sub(gv[:, :, 0, 1], m1[:, :], m2[:, :])
                    m3 = sc.tile([P, F], f32, tag="mA")
                    m4 = sc.tile([P, F], f32, tag="mB")
                    nc.vector.tensor_mul(m3[:, :], z2c[:, :, 2], gv[:, :, 0, 0])
                    nc.vector.tensor_mul(m4[:, :], z2c[:, :, 0], gv[:, :, 2, 0])
                    nc.vector.tensor_sub(gv[:, :, 1, 1], m3[:, :], m4[:, :])
                    m5 = sc.tile([P, F], f32, tag="mA")
                    m6 = sc.tile([P, F], f32, tag="mB")
                    nc.vector.tensor_mul(m5[:, :], z2c[:, :, 0], gv[:, :, 1, 0])
                    nc.vector.tensor_mul(m6[:, :], z2c[:, :, 1], gv[:, :, 0, 0])
                    nc.vector.tensor_sub(gv[:, :, 2, 1], m5[:, :], m6[:, :])
                    nc.vector.tensor_mul(ycol, ycol, bc3(rp[:, :]))

                    # ---- t column (constant bottom row pre-filled above) ----
                    nc.scalar.activation(tcol, cpc, Cpy)

                    # ---- stores ----
                    nc.sync.dma_start(out=grv[i], in_=g[:, :])
                    nc.sync.dma_start(out=ptv[i], in_=cpt[:, :])
    nc.finalize()
    return nc


_NC_CACHE = {}


def _get_nc(m):
    if m not in _NC_CACHE:
        _NC_CACHE[m] = build_nc(m)
    return _NC_CACHE[m]


def kernel(contact_pts, z1, z2, s_logits, w_logits, num_batches, _trace=False):
    n = contact_pts.shape[0]
    m = n // N_CORES
    nc = _get_nc(m)

    cp = np.ascontiguousarray(contact_pts, dtype=np.float32)
    z1 = np.ascontiguousarray(z1, dtype=np.float32)
    z2 = np.ascontiguousarray(z2, dtype=np.float32)
    sl = np.ascontiguousarray(s_logits, dtype=np.float32).reshape(n)
    wl = np.ascontiguousarray(w_logits, dtype=np.float32).reshape(n)

    in_maps = []
    for c in range(N_CORES):
        s0, s1 = c * m, (c + 1) * m
        in_maps.append({
            "z1": z1[s0:s1],
            "z2": z2[s0:s1],
            "cp": cp[s0:s1],
            "sl": sl[s0:s1],
            "wl": wl[s0:s1],
        })

    res = run_bass_kernel_spmd(nc, in_maps, list(range(N_CORES)), trace=_trace)

    points = np.concatenate([res.results[c]["pt"] for c in range(N_CORES)], axis=0)
    grasps = np.concatenate([res.results[c]["gr"] for c in range(N_CORES)], axis=0)
    s = np.concatenate([res.results[c]["so"] for c in range(N_CORES)], axis=0)
    w = np.concatenate([res.results[c]["wo"] for c in range(N_CORES)], axis=0)

    b = int(num_batches)
    p = n // b
    out = (
        points.reshape(b, p, 3),
        grasps.reshape(b, p, 4, 4),
        s.reshape(b, p),
        w.reshape(b, p),
    )
    if _trace:
        return out, res
    return out
